# revision 10
# baseline (speedup 1.0000x reference)
"""Trainium2 Bass kernel for nn_Aggregator (GNN message passing).

Computation (per batch b, entity e):
    scores[b,e,n]  = sum_d user[b,d] * rel[b,e,n,d]
    attn           = masked_softmax(scores)         (exp, zero where score==0,
                                                     guard zero denom)
    agg[b,e,d]     = sum_n attn[b,e,n] * nv[b,e,n,d]
    out            = relu((self[b,e,:] + agg[b,e,:]) @ W.T + b)

Sharding: pure data parallel over the batch dim B=1024 across 8 NeuronCores
(128 batches per core).  W/b replicated.  `masks` is all-ones and unused by
the reference computation, so it is never transferred.

Per-core layout: tiles of 2 batches -> SBUF tiles [128 part = (2b x 64e),
free = (32n x 64d)].  VectorE does the two big multiplies and segmented
reductions, ScalarE does exp/relu/copies, TensorE does the final 64x64
linear via a transpose + matmul (bias folded in as a rank-1 matmul).
"""

import sys

sys.path.insert(0, "/opt/trn_rl_repo")

from contextlib import ExitStack

import numpy as np

import concourse.bass as bass
import concourse.tile as tile
from concourse import bacc, mybir
from concourse.bass_utils import run_bass_kernel_spmd
from concourse.masks import make_identity

B, E, N, D = 1024, 64, 32, 64
N_CORES = 8
BC = B // N_CORES          # batches per core = 128
TB = 2                     # batches per tile
NTILES = BC // TB          # 64
P = TB * E                 # 128 partitions = (2 b, 64 e)

FP32 = mybir.dt.float32
BF16 = mybir.dt.bfloat16
Alu = mybir.AluOpType
Act = mybir.ActivationFunctionType
AxX = mybir.AxisListType.X

_CACHE = {}


def _build_kernel():
    nc = bacc.Bacc("TRN2", target_bir_lowering=False, debug=False)

    rel_d = nc.dram_tensor("rel", [BC, E, N, D], FP32, kind="ExternalInput")
    nv_d = nc.dram_tensor("nv", [BC, E, N, D], FP32, kind="ExternalInput")
    self_d = nc.dram_tensor("selfv", [BC, E, D], FP32, kind="ExternalInput")
    u_d = nc.dram_tensor("ue", [BC, D], FP32, kind="ExternalInput")
    w_d = nc.dram_tensor("w", [D, D], FP32, kind="ExternalInput")
    b_d = nc.dram_tensor("bias", [1, D], FP32, kind="ExternalInput")
    out_d = nc.dram_tensor("out", [BC, E, D], FP32, kind="ExternalOutput")

    rel_ap = rel_d.ap().rearrange("b e n d -> (b e) n d")
    nv_ap = nv_d.ap().rearrange("b e n d -> (b e) n d")
    self_ap = self_d.ap().rearrange("b e d -> (b e) d")
    out_ap = out_d.ap().rearrange("b e d -> (b e) d")

    with tile.TileContext(nc) as tc:
        with ExitStack() as ctx:
            singles = ctx.enter_context(tc.tile_pool(name="singles", bufs=1))
            pair = ctx.enter_context(tc.tile_pool(name="pair", bufs=2))
            big = ctx.enter_context(tc.tile_pool(name="big", bufs=3))
            small = ctx.enter_context(tc.tile_pool(name="small", bufs=4))
            outp = ctx.enter_context(tc.tile_pool(name="outp", bufs=4))
            psum = ctx.enter_context(tc.tile_pool(name="psum", bufs=3, space="PSUM"))

            # ---- constants ----
            ident = singles.tile([128, 128], FP32)
            make_identity(nc, ident[:])

            # u_all[p=(bo,e), i, d] = ue[2i+bo, d] — all per-tile user-emb
            # broadcasts materialized once (two stride-0-source DMAs).
            u_all = singles.tile([P, NTILES, D], BF16)
            for bo in range(TB):
                src = bass.AP(
                    tensor=u_d.ap().tensor,
                    offset=bo * D,
                    ap=[[0, E], [TB * D, NTILES], [1, D]],
                )
                nc.gpsimd.dma_start(u_all[bo * E : (bo + 1) * E, :, :], src)

            w_nat = singles.tile([D, D], FP32)
            nc.sync.dma_start(w_nat[:], w_d.ap()[:])
            wt_ps = psum.tile([D, D], FP32, tag="y")
            nc.tensor.transpose(wt_ps[:], w_nat[:], ident[0:D, 0:D])
            wt = singles.tile([D, D], FP32)          # wt[d, j] = W[j, d]
            nc.scalar.copy(wt[:], wt_ps[:])

            b_row = singles.tile([1, D], FP32)
            nc.sync.dma_start(b_row[:], b_d.ap()[:])
            ones_row = singles.tile([1, P], FP32)
            nc.vector.memset(ones_row[:], 1.0)

            # ---- main loop: pairs of 2-batch tiles (one 2 MiB DMA each) ----
            for j in range(NTILES // 2):
                q0 = j * 2 * P                        # first row (b*E) of pair

                rel2 = pair.tile([P, 2, N, D], BF16, tag="rel")
                nc.gpsimd.dma_start(
                    rel2[:],
                    bass.AP(
                        tensor=rel_ap.tensor,
                        offset=q0 * N * D,
                        ap=[[N * D, P], [P * N * D, 2], [D, N], [1, D]],
                    ),
                )
                nv2 = pair.tile([P, 2, N, D], FP32, tag="nv")
                nc.scalar.dma_start(
                    nv2[:],
                    bass.AP(
                        tensor=nv_ap.tensor,
                        offset=q0 * N * D,
                        ap=[[N * D, P], [P * N * D, 2], [D, N], [1, D]],
                    ),
                )
                self2 = small.tile([P, 2, D], FP32, tag="self")
                nc.sync.dma_start(
                    self2[:],
                    bass.AP(
                        tensor=self_ap.tensor,
                        offset=q0 * D,
                        ap=[[D, P], [P * D, 2], [1, D]],
                    ),
                )
                out2 = outp.tile([P, 2, D], FP32, tag="out")

                for t in range(2):
                    i = 2 * j + t                     # logical 2-batch tile
                    rel_t = rel2[:, t]
                    nv_t = nv2[:, t]
                    self_t = self2[:, t]

                    # scores[p, n] = sum_d rel[p, n, d] * u[p, d]
                    prod = big.tile([P, N, D], BF16, tag="prod")
                    nc.vector.tensor_mul(
                        prod[:],
                        rel_t,
                        u_all[:, i : i + 1, :].broadcast_to((P, N, D)),
                    )
                    scores = small.tile([P, N], FP32, tag="scores")
                    nc.vector.reduce_sum(scores[:], prod[:], axis=AxX)

                    # masked softmax (defer the 1/denom to the combine step)
                    e_t = small.tile([P, N], FP32, tag="e")
                    nc.scalar.activation(e_t[:], scores[:], Act.Exp)
                    em = small.tile([P, N], FP32, tag="em")
                    ssum = small.tile([P, 1], FP32, tag="ssum")
                    nc.vector.scalar_tensor_tensor(
                        out=em[:],
                        in0=scores[:],
                        scalar=0.0,
                        in1=e_t[:],
                        op0=Alu.not_equal,
                        op1=Alu.mult,
                        accum_out=ssum[:],
                    )
                    denom = small.tile([P, 1], FP32, tag="denom")
                    nc.vector.scalar_tensor_tensor(
                        out=denom[:],
                        in0=ssum[:],
                        scalar=0.0,
                        in1=ssum[:],
                        op0=Alu.is_equal,
                        op1=Alu.add,
                    )
                    recip = small.tile([P, 1], FP32, tag="recip")
                    nc.vector.reciprocal(recip[:], denom[:])

                    # agg[p, d] = sum_n em[p, n] * nv[p, n, d]
                    # DVE multiply with natural reads and a d-major (strided)
                    # write, so the segmented n-reduce reads contiguously.
                    prod2 = big.tile([P, D, N], FP32, tag="prod2")
                    nc.vector.tensor_mul(
                        prod2[:].transpose([0, 2, 1]),
                        nv_t,
                        em[:].unsqueeze(2).broadcast_to((P, N, D)),
                    )
                    agg = small.tile([P, D], FP32, tag="agg")
                    nc.vector.reduce_sum(agg[:], prod2[:], axis=AxX)

                    # x = agg * (1/denom) + self
                    x_t = small.tile([P, D], FP32, tag="x")
                    nc.vector.scalar_tensor_tensor(
                        out=x_t[:],
                        in0=agg[:],
                        scalar=recip[:],
                        in1=self_t,
                        op0=Alu.mult,
                        op1=Alu.add,
                    )

                    # out = relu(x @ W.T + b) via PE
                    xT_ps = psum.tile([D, P], FP32, tag="xT")
                    nc.tensor.transpose(xT_ps[:], x_t[:], ident[:])
                    xT = small.tile([D, P], FP32, tag="xTs")
                    nc.scalar.copy(xT[:], xT_ps[:])
                    y_ps = psum.tile([P, D], FP32, tag="y")
                    nc.tensor.matmul(
                        y_ps[:], xT[:], wt[:], start=True, stop=False
                    )
                    nc.tensor.matmul(
                        y_ps[:], ones_row[:], b_row[:], start=False, stop=True
                    )
                    nc.scalar.activation(out2[:, t], y_ps[:], Act.Relu)

                nc.sync.dma_start(
                    bass.AP(
                        tensor=out_ap.tensor,
                        offset=q0 * D,
                        ap=[[D, P], [P * D, 2], [1, D]],
                    ),
                    out2[:],
                )

    nc.compile()
    return nc


def get_nc():
    if "nc" not in _CACHE:
        _CACHE["nc"] = _build_kernel()
    return _CACHE["nc"]


def kernel(
    self_vectors,
    neighbor_vectors,
    neighbor_relations,
    masks,
    user_embeddings,
    W,
    b,
    **_unused,
):
    del masks  # all-ones and unused by the reference computation
    nc = get_nc()

    self_v = np.ascontiguousarray(
        np.asarray(self_vectors, dtype=np.float32).reshape(B, E, D)
    )
    nv = np.ascontiguousarray(np.asarray(neighbor_vectors, dtype=np.float32))
    rel = np.ascontiguousarray(np.asarray(neighbor_relations, dtype=np.float32))
    ue = np.ascontiguousarray(np.asarray(user_embeddings, dtype=np.float32))
    w = np.ascontiguousarray(np.asarray(W, dtype=np.float32))
    bias = np.ascontiguousarray(np.asarray(b, dtype=np.float32).reshape(1, D))

    in_maps = []
    for c in range(N_CORES):
        s = slice(c * BC, (c + 1) * BC)
        in_maps.append(
            {
                "rel": rel[s],
                "nv": nv[s],
                "selfv": self_v[s],
                "ue": ue[s],
                "w": w,
                "bias": bias,
            }
        )

    res = run_bass_kernel_spmd(nc, in_maps, core_ids=list(range(N_CORES)))
    out = np.concatenate([res.results[c]["out"] for c in range(N_CORES)], axis=0)
    return out.reshape(B, E, D)


def run_traced(**inputs):
    """Like kernel() but also returns the BassKernelResults (with trace)."""
    nc = get_nc()
    self_v = np.asarray(inputs["self_vectors"], dtype=np.float32).reshape(B, E, D)
    nv = np.asarray(inputs["neighbor_vectors"], dtype=np.float32)
    rel = np.asarray(inputs["neighbor_relations"], dtype=np.float32)
    ue = np.asarray(inputs["user_embeddings"], dtype=np.float32)
    w = np.asarray(inputs["W"], dtype=np.float32)
    bias = np.asarray(inputs["b"], dtype=np.float32).reshape(1, D)
    in_maps = []
    for c in range(N_CORES):
        s = slice(c * BC, (c + 1) * BC)
        in_maps.append(
            {
                "rel": np.ascontiguousarray(rel[s]),
                "nv": np.ascontiguousarray(nv[s]),
                "selfv": np.ascontiguousarray(self_v[s]),
                "ue": np.ascontiguousarray(ue[s]),
                "w": np.ascontiguousarray(w),
                "bias": np.ascontiguousarray(bias),
            }
        )
    res = run_bass_kernel_spmd(
        nc, in_maps, core_ids=list(range(N_CORES)), trace=True
    )
    out = np.concatenate([res.results[c]["out"] for c in range(N_CORES)], axis=0)
    return out.reshape(B, E, D), res


# revision 12
# speedup vs baseline: 1.3513x; 1.3513x over previous
"""Trainium2 Bass kernel for nn_Aggregator (GNN message passing).

Computation (per batch b, entity e):
    scores[b,e,n]  = sum_d user[b,d] * rel[b,e,n,d]
    attn           = masked_softmax(scores)         (exp, zero where score==0,
                                                     guard zero denom)
    agg[b,e,d]     = sum_n attn[b,e,n] * nv[b,e,n,d]
    out            = relu((self[b,e,:] + agg[b,e,:]) @ W.T + b)

Sharding: pure data parallel over the batch dim B=1024 across 8 NeuronCores
(128 batches per core).  W/b replicated.  `masks` is all-ones and unused by
the reference computation, so it is never transferred.

Per-core layout: tiles of 2 batches -> SBUF tiles [128 part = (2b x 64e),
free = (32n x 64d)].  VectorE does the two big multiplies and segmented
reductions, ScalarE does exp/relu/copies, TensorE does the final 64x64
linear via a transpose + matmul (bias folded in as a rank-1 matmul).
"""

import sys

sys.path.insert(0, "/opt/trn_rl_repo")

from contextlib import ExitStack

import numpy as np

import concourse.bass as bass
import concourse.tile as tile
from concourse import bacc, mybir
from concourse.bass_utils import run_bass_kernel_spmd
from concourse.masks import make_identity

# ---- custom fused DVE op: out = cumsum(in0 * in1) along the free stream ----
# Segment sums (the mul+segmented-reduce fusion this kernel needs) are then
# read off as differences of segment-end prefixes with tiny strided ops.
import concourse.dve_ops as _dops
from concourse.dve_spec import Spec as _Spec, Src0 as _Src0, Src1 as _Src1, \
    AluOp as _DveAlu, scan as _dve_scan, lower as _dve_lower, \
    _has_src1 as _dve_has_src1
from concourse.dve_uop import DveOpSpec as _DveOpSpec


def _register_mulcumsum():
    name = "ANT_MUL_CUMSUM_69200513"
    if name in _dops.CUSTOM_DVE_SPECS:
        return _dops_by_name(name)

    def _ref(in0, in1, s0, s1, imm2):
        import numpy as _np

        pdim = in0.shape[0]
        a = _np.asarray(in0, _np.float32).reshape(pdim, -1)
        b = _np.asarray(in1, _np.float32).reshape(pdim, -1)
        return _np.cumsum(a * b, axis=-1, dtype=_np.float32)

    spec = _Spec(
        body=_dve_scan(_DveAlu.ADD, _Src0 * _Src1),
        reference=_ref,
    )
    row = len(_dops.OPS) + 1          # _CUSTOM_DVE_ROW_BASE + index
    shas = {}
    for ver in ("v3", "v4"):
        try:
            uops = _dve_lower(spec, ver=ver)
        except Exception:
            continue
        shas[ver] = _DveOpSpec(
            name=name, opcode=row, uops=uops, rd1_en=_dve_has_src1(spec)
        ).sha(ver)
    op = _dops.DveOp(name, spec, subdim=False, uops_sha=shas)
    _dops.OPS.append(op)
    _dops.CUSTOM_DVE_SPECS[name] = spec
    _dops._SUB_OPCODE_FOR_NAME[name] = row
    return op


def _dops_by_name(name):
    for o in _dops.OPS:
        if o.name == name:
            return o
    raise KeyError(name)


MUL_CUMSUM = _register_mulcumsum()

B, E, N, D = 1024, 64, 32, 64
N_CORES = 8
BC = B // N_CORES          # batches per core = 128
TB = 2                     # batches per tile
NTILES = BC // TB          # 64
P = TB * E                 # 128 partitions = (2 b, 64 e)

FP32 = mybir.dt.float32
BF16 = mybir.dt.bfloat16
Alu = mybir.AluOpType
Act = mybir.ActivationFunctionType
AxX = mybir.AxisListType.X

_CACHE = {}


def _build_kernel():
    nc = bacc.Bacc("TRN2", target_bir_lowering=False, debug=False)

    rel_d = nc.dram_tensor("rel", [BC, E, N, D], FP32, kind="ExternalInput")
    nv_d = nc.dram_tensor("nv", [BC, E, N, D], FP32, kind="ExternalInput")
    self_d = nc.dram_tensor("selfv", [BC, E, D], FP32, kind="ExternalInput")
    u_d = nc.dram_tensor("ue", [BC, D], FP32, kind="ExternalInput")
    w_d = nc.dram_tensor("w", [D, D], FP32, kind="ExternalInput")
    b_d = nc.dram_tensor("bias", [1, D], FP32, kind="ExternalInput")
    out_d = nc.dram_tensor("out", [BC, E, D], FP32, kind="ExternalOutput")

    rel_ap = rel_d.ap().rearrange("b e n d -> (b e) n d")
    nv_ap = nv_d.ap().rearrange("b e n d -> (b e) n d")
    self_ap = self_d.ap().rearrange("b e d -> (b e) d")
    out_ap = out_d.ap().rearrange("b e d -> (b e) d")

    with tile.TileContext(nc) as tc:
        with ExitStack() as ctx:
            singles = ctx.enter_context(tc.tile_pool(name="singles", bufs=1))
            pair = ctx.enter_context(tc.tile_pool(name="pair", bufs=2))
            big = ctx.enter_context(tc.tile_pool(name="big", bufs=3))
            small = ctx.enter_context(tc.tile_pool(name="small", bufs=4))
            outp = ctx.enter_context(tc.tile_pool(name="outp", bufs=4))
            psum = ctx.enter_context(tc.tile_pool(name="psum", bufs=3, space="PSUM"))

            # ---- constants ----
            ident = singles.tile([128, 128], FP32)
            make_identity(nc, ident[:])

            # u_all[p=(bo,e), i, d] = ue[2i+bo, d] — all per-tile user-emb
            # broadcasts materialized once (two stride-0-source DMAs).
            u_all = singles.tile([P, NTILES, D], FP32)
            for bo in range(TB):
                src = bass.AP(
                    tensor=u_d.ap().tensor,
                    offset=bo * D,
                    ap=[[0, E], [TB * D, NTILES], [1, D]],
                )
                nc.gpsimd.dma_start(u_all[bo * E : (bo + 1) * E, :, :], src)

            w_nat = singles.tile([D, D], FP32)
            nc.sync.dma_start(w_nat[:], w_d.ap()[:])
            wt_ps = psum.tile([D, D], FP32, tag="y")
            nc.tensor.transpose(wt_ps[:], w_nat[:], ident[0:D, 0:D])
            wt = singles.tile([D, D], FP32)          # wt[d, j] = W[j, d]
            nc.scalar.copy(wt[:], wt_ps[:])

            b_row = singles.tile([1, D], FP32)
            nc.sync.dma_start(b_row[:], b_d.ap()[:])
            ones_row = singles.tile([1, P], FP32)
            nc.vector.memset(ones_row[:], 1.0)

            # ---- main loop: pairs of 2-batch tiles (one 2 MiB DMA each) ----
            for j in range(NTILES // 2):
                q0 = j * 2 * P                        # first row (b*E) of pair

                rel2 = pair.tile([P, 2, N, D], FP32, tag="rel")
                nc.sync.dma_start(
                    rel2[:],
                    bass.AP(
                        tensor=rel_ap.tensor,
                        offset=q0 * N * D,
                        ap=[[N * D, P], [P * N * D, 2], [D, N], [1, D]],
                    ),
                )
                nv2 = pair.tile([P, 2, N, D], FP32, tag="nv")
                nc.scalar.dma_start(
                    nv2[:],
                    bass.AP(
                        tensor=nv_ap.tensor,
                        offset=q0 * N * D,
                        ap=[[N * D, P], [P * N * D, 2], [D, N], [1, D]],
                    ),
                )
                self2 = small.tile([P, 2, D], FP32, tag="self")
                nc.sync.dma_start(
                    self2[:],
                    bass.AP(
                        tensor=self_ap.tensor,
                        offset=q0 * D,
                        ap=[[D, P], [P * D, 2], [1, D]],
                    ),
                )
                out2 = outp.tile([P, 2, D], FP32, tag="out")

                for t in range(2):
                    i = 2 * j + t                     # logical 2-batch tile
                    rel_t = rel2[:, t]
                    nv_t = nv2[:, t]
                    self_t = self2[:, t]

                    # scores[p, n] = sum_d rel[p, n, d] * u[p, d]
                    # One fused pass: cum = cumsum(rel * u_bcast); the per-n
                    # sums are differences of segment-end prefixes.
                    cum = big.tile([P, N, D], FP32, tag="prod")
                    nc.vector._custom_dve(
                        MUL_CUMSUM,
                        out=cum[:],
                        in0=rel_t,
                        in1=u_all[:, i : i + 1, :].broadcast_to((P, N, D)),
                    )
                    scores = small.tile([P, N], FP32, tag="scores")
                    nc.vector.tensor_copy(scores[:, 0:1], cum[:, 0:1, D - 1])
                    nc.vector.tensor_sub(
                        scores[:, 1:N], cum[:, 1:N, D - 1], cum[:, 0 : N - 1, D - 1]
                    )

                    # masked softmax (defer the 1/denom to the combine step)
                    e_t = small.tile([P, N], FP32, tag="e")
                    nc.scalar.activation(e_t[:], scores[:], Act.Exp)
                    em = small.tile([P, N], FP32, tag="em")
                    ssum = small.tile([P, 1], FP32, tag="ssum")
                    nc.vector.scalar_tensor_tensor(
                        out=em[:],
                        in0=scores[:],
                        scalar=0.0,
                        in1=e_t[:],
                        op0=Alu.not_equal,
                        op1=Alu.mult,
                        accum_out=ssum[:],
                    )
                    denom = small.tile([P, 1], FP32, tag="denom")
                    nc.vector.scalar_tensor_tensor(
                        out=denom[:],
                        in0=ssum[:],
                        scalar=0.0,
                        in1=ssum[:],
                        op0=Alu.is_equal,
                        op1=Alu.add,
                    )
                    recip = small.tile([P, 1], FP32, tag="recip")
                    nc.vector.reciprocal(recip[:], denom[:])

                    # agg[p, d] = sum_n em[p, n] * nv[p, n, d]
                    # Same fused pass over the d-major stream: cum2 = cumsum
                    # over (d, n) of em * nv; agg comes from segment-end diffs.
                    cum2 = big.tile([P, D, N], FP32, tag="prod2")
                    nc.vector._custom_dve(
                        MUL_CUMSUM,
                        out=cum2[:],
                        in0=nv_t.transpose([0, 2, 1]),
                        in1=em[:].unsqueeze(1).broadcast_to((P, D, N)),
                    )
                    agg = small.tile([P, D], FP32, tag="agg")
                    nc.vector.tensor_copy(agg[:, 0:1], cum2[:, 0:1, N - 1])
                    nc.vector.tensor_sub(
                        agg[:, 1:D], cum2[:, 1:D, N - 1], cum2[:, 0 : D - 1, N - 1]
                    )

                    # x = agg * (1/denom) + self
                    x_t = small.tile([P, D], FP32, tag="x")
                    nc.vector.scalar_tensor_tensor(
                        out=x_t[:],
                        in0=agg[:],
                        scalar=recip[:],
                        in1=self_t,
                        op0=Alu.mult,
                        op1=Alu.add,
                    )

                    # out = relu(x @ W.T + b) via PE
                    xT_ps = psum.tile([D, P], FP32, tag="xT")
                    nc.tensor.transpose(xT_ps[:], x_t[:], ident[:])
                    xT = small.tile([D, P], FP32, tag="xTs")
                    nc.scalar.copy(xT[:], xT_ps[:])
                    y_ps = psum.tile([P, D], FP32, tag="y")
                    nc.tensor.matmul(
                        y_ps[:], xT[:], wt[:], start=True, stop=False
                    )
                    nc.tensor.matmul(
                        y_ps[:], ones_row[:], b_row[:], start=False, stop=True
                    )
                    nc.scalar.activation(out2[:, t], y_ps[:], Act.Relu)

                nc.sync.dma_start(
                    bass.AP(
                        tensor=out_ap.tensor,
                        offset=q0 * D,
                        ap=[[D, P], [P * D, 2], [1, D]],
                    ),
                    out2[:],
                )

    nc.compile()
    return nc


def get_nc():
    if "nc" not in _CACHE:
        _CACHE["nc"] = _build_kernel()
    return _CACHE["nc"]


def kernel(
    self_vectors,
    neighbor_vectors,
    neighbor_relations,
    masks,
    user_embeddings,
    W,
    b,
    **_unused,
):
    del masks  # all-ones and unused by the reference computation
    nc = get_nc()

    self_v = np.ascontiguousarray(
        np.asarray(self_vectors, dtype=np.float32).reshape(B, E, D)
    )
    nv = np.ascontiguousarray(np.asarray(neighbor_vectors, dtype=np.float32))
    rel = np.ascontiguousarray(np.asarray(neighbor_relations, dtype=np.float32))
    ue = np.ascontiguousarray(np.asarray(user_embeddings, dtype=np.float32))
    w = np.ascontiguousarray(np.asarray(W, dtype=np.float32))
    bias = np.ascontiguousarray(np.asarray(b, dtype=np.float32).reshape(1, D))

    in_maps = []
    for c in range(N_CORES):
        s = slice(c * BC, (c + 1) * BC)
        in_maps.append(
            {
                "rel": rel[s],
                "nv": nv[s],
                "selfv": self_v[s],
                "ue": ue[s],
                "w": w,
                "bias": bias,
            }
        )

    res = run_bass_kernel_spmd(nc, in_maps, core_ids=list(range(N_CORES)))
    out = np.concatenate([res.results[c]["out"] for c in range(N_CORES)], axis=0)
    return out.reshape(B, E, D)


def run_traced(**inputs):
    """Like kernel() but also returns the BassKernelResults (with trace)."""
    nc = get_nc()
    self_v = np.asarray(inputs["self_vectors"], dtype=np.float32).reshape(B, E, D)
    nv = np.asarray(inputs["neighbor_vectors"], dtype=np.float32)
    rel = np.asarray(inputs["neighbor_relations"], dtype=np.float32)
    ue = np.asarray(inputs["user_embeddings"], dtype=np.float32)
    w = np.asarray(inputs["W"], dtype=np.float32)
    bias = np.asarray(inputs["b"], dtype=np.float32).reshape(1, D)
    in_maps = []
    for c in range(N_CORES):
        s = slice(c * BC, (c + 1) * BC)
        in_maps.append(
            {
                "rel": np.ascontiguousarray(rel[s]),
                "nv": np.ascontiguousarray(nv[s]),
                "selfv": np.ascontiguousarray(self_v[s]),
                "ue": np.ascontiguousarray(ue[s]),
                "w": np.ascontiguousarray(w),
                "bias": np.ascontiguousarray(bias),
            }
        )
    res = run_bass_kernel_spmd(
        nc, in_maps, core_ids=list(range(N_CORES)), trace=True
    )
    out = np.concatenate([res.results[c]["out"] for c in range(N_CORES)], axis=0)
    return out.reshape(B, E, D), res


# revision 14
# speedup vs baseline: 1.6000x; 1.1841x over previous
"""Trainium2 Bass kernel for nn_Aggregator (GNN message passing).

Computation (per batch b, entity e):
    scores[b,e,n]  = sum_d user[b,d] * rel[b,e,n,d]
    attn           = masked_softmax(scores)         (exp, zero where score==0,
                                                     guard zero denom)
    agg[b,e,d]     = sum_n attn[b,e,n] * nv[b,e,n,d]
    out            = relu((self[b,e,:] + agg[b,e,:]) @ W.T + b)

Sharding: pure data parallel over the batch dim B=1024 across 8 NeuronCores
(128 batches per core).  W/b replicated.  `masks` is all-ones and unused by
the reference computation, so it is never transferred.

Per-core layout: tiles of 2 batches -> SBUF tiles [128 part = (2b x 64e),
free = (32n x 64d)].  VectorE does the two big multiplies and segmented
reductions, ScalarE does exp/relu/copies, TensorE does the final 64x64
linear via a transpose + matmul (bias folded in as a rank-1 matmul).
"""

import sys

sys.path.insert(0, "/opt/trn_rl_repo")

from contextlib import ExitStack

import numpy as np

import concourse.bass as bass
import concourse.tile as tile
from concourse import bacc, mybir
from concourse.bass_utils import run_bass_kernel_spmd
from concourse.masks import make_identity

# ---- custom fused DVE op: out = cumsum(in0 * in1) along the free stream ----
# Segment sums (the mul+segmented-reduce fusion this kernel needs) are then
# read off as differences of segment-end prefixes with tiny strided ops.
import concourse.dve_ops as _dops
from concourse.dve_spec import Spec as _Spec, Src0 as _Src0, Src1 as _Src1, \
    AluOp as _DveAlu, scan as _dve_scan, lower as _dve_lower, \
    _has_src1 as _dve_has_src1
from concourse.dve_uop import DveOpSpec as _DveOpSpec


def _register_mulcumsum():
    name = "ANT_MUL_CUMSUM_69200513"
    if name in _dops.CUSTOM_DVE_SPECS:
        return _dops_by_name(name)

    def _ref(in0, in1, s0, s1, imm2):
        import numpy as _np

        pdim = in0.shape[0]
        a = _np.asarray(in0, _np.float32).reshape(pdim, -1)
        b = _np.asarray(in1, _np.float32).reshape(pdim, -1)
        return _np.cumsum(a * b, axis=-1, dtype=_np.float32)

    spec = _Spec(
        body=_dve_scan(_DveAlu.ADD, _Src0 * _Src1),
        reference=_ref,
    )
    row = len(_dops.OPS) + 1          # _CUSTOM_DVE_ROW_BASE + index
    shas = {}
    for ver in ("v3", "v4"):
        try:
            uops = _dve_lower(spec, ver=ver)
        except Exception:
            continue
        shas[ver] = _DveOpSpec(
            name=name, opcode=row, uops=uops, rd1_en=_dve_has_src1(spec)
        ).sha(ver)
    op = _dops.DveOp(name, spec, subdim=False, uops_sha=shas)
    _dops.OPS.append(op)
    _dops.CUSTOM_DVE_SPECS[name] = spec
    _dops._SUB_OPCODE_FOR_NAME[name] = row
    return op


def _dops_by_name(name):
    for o in _dops.OPS:
        if o.name == name:
            return o
    raise KeyError(name)


MUL_CUMSUM = _register_mulcumsum()

B, E, N, D = 1024, 64, 32, 64
N_CORES = 8
BC = B // N_CORES          # batches per core = 128
TB = 2                     # batches per tile
NTILES = BC // TB          # 64
P = TB * E                 # 128 partitions = (2 b, 64 e)

FP32 = mybir.dt.float32
BF16 = mybir.dt.bfloat16
Alu = mybir.AluOpType
Act = mybir.ActivationFunctionType
AxX = mybir.AxisListType.X

_CACHE = {}


def _build_kernel():
    nc = bacc.Bacc("TRN2", target_bir_lowering=False, debug=False)

    rel_d = nc.dram_tensor("rel", [BC, E, N, D], FP32, kind="ExternalInput")
    nv_d = nc.dram_tensor("nv", [BC, E, N, D], FP32, kind="ExternalInput")
    self_d = nc.dram_tensor("selfv", [BC, E, D], FP32, kind="ExternalInput")
    u_d = nc.dram_tensor("ue", [BC, D], FP32, kind="ExternalInput")
    w_d = nc.dram_tensor("w", [D, D], FP32, kind="ExternalInput")
    b_d = nc.dram_tensor("bias", [1, D], FP32, kind="ExternalInput")
    out_d = nc.dram_tensor("out", [BC, E, D], FP32, kind="ExternalOutput")

    rel_ap = rel_d.ap().rearrange("b e n d -> (b e) n d")
    nv_ap = nv_d.ap().rearrange("b e n d -> (b e) n d")
    self_ap = self_d.ap().rearrange("b e d -> (b e) d")
    out_ap = out_d.ap().rearrange("b e d -> (b e) d")

    with tile.TileContext(nc) as tc:
        with ExitStack() as ctx:
            singles = ctx.enter_context(tc.tile_pool(name="singles", bufs=1))
            pair = ctx.enter_context(tc.tile_pool(name="pair", bufs=3))
            big = ctx.enter_context(tc.tile_pool(name="big", bufs=3))
            small = ctx.enter_context(tc.tile_pool(name="small", bufs=4))
            outp = ctx.enter_context(tc.tile_pool(name="outp", bufs=4))
            psum = ctx.enter_context(tc.tile_pool(name="psum", bufs=3, space="PSUM"))

            # ---- constants ----
            ident = singles.tile([128, 128], FP32)
            make_identity(nc, ident[:])

            # u_all[p=(bo,e), i, d] = ue[2i+bo, d] — all per-tile user-emb
            # broadcasts materialized once (two stride-0-source DMAs).
            u_all = singles.tile([P, NTILES, D], FP32)
            for bo in range(TB):
                src = bass.AP(
                    tensor=u_d.ap().tensor,
                    offset=bo * D,
                    ap=[[0, E], [TB * D, NTILES], [1, D]],
                )
                nc.gpsimd.dma_start(u_all[bo * E : (bo + 1) * E, :, :], src)

            w_nat = singles.tile([D, D], FP32)
            nc.sync.dma_start(w_nat[:], w_d.ap()[:])
            wt_ps = psum.tile([D, D], FP32, tag="y")
            nc.tensor.transpose(wt_ps[:], w_nat[:], ident[0:D, 0:D])
            wt = singles.tile([D, D], FP32)          # wt[d, j] = W[j, d]
            nc.scalar.copy(wt[:], wt_ps[:])

            b_row = singles.tile([1, D], FP32)
            nc.sync.dma_start(b_row[:], b_d.ap()[:])
            ones_row = singles.tile([1, P], FP32)
            nc.vector.memset(ones_row[:], 1.0)

            # ---- main loop: pairs of 2-batch tiles (one 2 MiB DMA each) ----
            for j in range(NTILES // 2):
                q0 = j * 2 * P                        # first row (b*E) of pair

                rel2 = pair.tile([P, 2, N, D], FP32, tag="rel")
                nc.sync.dma_start(
                    rel2[:],
                    bass.AP(
                        tensor=rel_ap.tensor,
                        offset=q0 * N * D,
                        ap=[[N * D, P], [P * N * D, 2], [D, N], [1, D]],
                    ),
                )
                nv2 = pair.tile([P, 2, N, D], FP32, tag="nv")
                nc.scalar.dma_start(
                    nv2[:],
                    bass.AP(
                        tensor=nv_ap.tensor,
                        offset=q0 * N * D,
                        ap=[[N * D, P], [P * N * D, 2], [D, N], [1, D]],
                    ),
                )
                self2 = small.tile([P, 2, D], FP32, tag="self")
                nc.gpsimd.dma_start(
                    self2[:],
                    bass.AP(
                        tensor=self_ap.tensor,
                        offset=q0 * D,
                        ap=[[D, P], [P * D, 2], [1, D]],
                    ),
                )
                out2 = outp.tile([P, 2, D], FP32, tag="out")

                for t in range(2):
                    i = 2 * j + t                     # logical 2-batch tile
                    rel_t = rel2[:, t]
                    nv_t = nv2[:, t]
                    self_t = self2[:, t]

                    # scores[p, n] = sum_d rel[p, n, d] * u[p, d]
                    # One fused pass: cum = cumsum(rel * u_bcast); the per-n
                    # sums are differences of segment-end prefixes.
                    cum = big.tile([P, N, D], FP32, tag="prod")
                    nc.vector._custom_dve(
                        MUL_CUMSUM,
                        out=cum[:],
                        in0=rel_t,
                        in1=u_all[:, i : i + 1, :].broadcast_to((P, N, D)),
                    )
                    scores = small.tile([P, N], FP32, tag="scores")
                    nc.scalar.copy(scores[:, 0:1], cum[:, 0:1, D - 1])
                    nc.gpsimd.tensor_sub(
                        scores[:, 1:N], cum[:, 1:N, D - 1], cum[:, 0 : N - 1, D - 1]
                    )

                    # masked softmax (defer the 1/denom to the combine step)
                    e_t = small.tile([P, N], FP32, tag="e")
                    nc.scalar.activation(e_t[:], scores[:], Act.Exp)
                    em = small.tile([P, N], FP32, tag="em")
                    ssum = small.tile([P, 1], FP32, tag="ssum")
                    nc.vector.scalar_tensor_tensor(
                        out=em[:],
                        in0=scores[:],
                        scalar=0.0,
                        in1=e_t[:],
                        op0=Alu.not_equal,
                        op1=Alu.mult,
                        accum_out=ssum[:],
                    )
                    denom = small.tile([P, 1], FP32, tag="denom")
                    nc.vector.scalar_tensor_tensor(
                        out=denom[:],
                        in0=ssum[:],
                        scalar=0.0,
                        in1=ssum[:],
                        op0=Alu.is_equal,
                        op1=Alu.add,
                    )
                    recip = small.tile([P, 1], FP32, tag="recip")
                    nc.vector.reciprocal(recip[:], denom[:])

                    # agg[p, d] = sum_n em[p, n] * nv[p, n, d]
                    # Same fused pass over the d-major stream: cum2 = cumsum
                    # over (d, n) of em * nv; agg comes from segment-end diffs.
                    cum2 = big.tile([P, D, N], FP32, tag="prod2")
                    nc.vector._custom_dve(
                        MUL_CUMSUM,
                        out=cum2[:],
                        in0=nv_t.transpose([0, 2, 1]),
                        in1=em[:].unsqueeze(1).broadcast_to((P, D, N)),
                    )
                    agg = small.tile([P, D], FP32, tag="agg")
                    nc.scalar.copy(agg[:, 0:1], cum2[:, 0:1, N - 1])
                    nc.gpsimd.tensor_sub(
                        agg[:, 1:D], cum2[:, 1:D, N - 1], cum2[:, 0 : D - 1, N - 1]
                    )

                    # x = agg * (1/denom) + self
                    x_t = small.tile([P, D], FP32, tag="x")
                    nc.vector.scalar_tensor_tensor(
                        out=x_t[:],
                        in0=agg[:],
                        scalar=recip[:],
                        in1=self_t,
                        op0=Alu.mult,
                        op1=Alu.add,
                    )

                    # out = relu(x @ W.T + b) via PE
                    xT_ps = psum.tile([D, P], FP32, tag="xT")
                    nc.tensor.transpose(xT_ps[:], x_t[:], ident[:])
                    xT = small.tile([D, P], FP32, tag="xTs")
                    nc.scalar.copy(xT[:], xT_ps[:])
                    y_ps = psum.tile([P, D], FP32, tag="y")
                    nc.tensor.matmul(
                        y_ps[:], xT[:], wt[:], start=True, stop=False
                    )
                    nc.tensor.matmul(
                        y_ps[:], ones_row[:], b_row[:], start=False, stop=True
                    )
                    nc.scalar.activation(out2[:, t], y_ps[:], Act.Relu)

                nc.gpsimd.dma_start(
                    bass.AP(
                        tensor=out_ap.tensor,
                        offset=q0 * D,
                        ap=[[D, P], [P * D, 2], [1, D]],
                    ),
                    out2[:],
                )

    nc.compile()
    return nc


def get_nc():
    if "nc" not in _CACHE:
        _CACHE["nc"] = _build_kernel()
    return _CACHE["nc"]


def kernel(
    self_vectors,
    neighbor_vectors,
    neighbor_relations,
    masks,
    user_embeddings,
    W,
    b,
    **_unused,
):
    del masks  # all-ones and unused by the reference computation
    nc = get_nc()

    self_v = np.ascontiguousarray(
        np.asarray(self_vectors, dtype=np.float32).reshape(B, E, D)
    )
    nv = np.ascontiguousarray(np.asarray(neighbor_vectors, dtype=np.float32))
    rel = np.ascontiguousarray(np.asarray(neighbor_relations, dtype=np.float32))
    ue = np.ascontiguousarray(np.asarray(user_embeddings, dtype=np.float32))
    w = np.ascontiguousarray(np.asarray(W, dtype=np.float32))
    bias = np.ascontiguousarray(np.asarray(b, dtype=np.float32).reshape(1, D))

    in_maps = []
    for c in range(N_CORES):
        s = slice(c * BC, (c + 1) * BC)
        in_maps.append(
            {
                "rel": rel[s],
                "nv": nv[s],
                "selfv": self_v[s],
                "ue": ue[s],
                "w": w,
                "bias": bias,
            }
        )

    res = run_bass_kernel_spmd(nc, in_maps, core_ids=list(range(N_CORES)))
    out = np.concatenate([res.results[c]["out"] for c in range(N_CORES)], axis=0)
    return out.reshape(B, E, D)


def run_traced(**inputs):
    """Like kernel() but also returns the BassKernelResults (with trace)."""
    nc = get_nc()
    self_v = np.asarray(inputs["self_vectors"], dtype=np.float32).reshape(B, E, D)
    nv = np.asarray(inputs["neighbor_vectors"], dtype=np.float32)
    rel = np.asarray(inputs["neighbor_relations"], dtype=np.float32)
    ue = np.asarray(inputs["user_embeddings"], dtype=np.float32)
    w = np.asarray(inputs["W"], dtype=np.float32)
    bias = np.asarray(inputs["b"], dtype=np.float32).reshape(1, D)
    in_maps = []
    for c in range(N_CORES):
        s = slice(c * BC, (c + 1) * BC)
        in_maps.append(
            {
                "rel": np.ascontiguousarray(rel[s]),
                "nv": np.ascontiguousarray(nv[s]),
                "selfv": np.ascontiguousarray(self_v[s]),
                "ue": np.ascontiguousarray(ue[s]),
                "w": np.ascontiguousarray(w),
                "bias": np.ascontiguousarray(bias),
            }
        )
    res = run_bass_kernel_spmd(
        nc, in_maps, core_ids=list(range(N_CORES)), trace=True
    )
    out = np.concatenate([res.results[c]["out"] for c in range(N_CORES)], axis=0)
    return out.reshape(B, E, D), res
